# revision 11
# baseline (speedup 1.0000x reference)
"""Trainium2 Bass kernel for nn_MultiHeadAttention_Linear_11312943857747.

Math (B=4, S=4096, DM=1024, H=16, HD=64):
    q = softmax(x @ Wq.T + bq) over head_dim
    k = softmax(x @ Wk.T + bk) over seq_len
    v = x @ Wv.T + bv
    gmap[b,h] = k[b,h].T @ v[b,h]            (HD x HD per head)
    o[b,h]    = q[b,h] @ gmap[b,h]
    out = LayerNorm(x + o) * gamma + beta

Sharding: 8 cores = 4 batches x 2 sequence-halves. Each core projects its
2048 rows; the per-head kT@v reduction over the full sequence is completed
with a tiny (132KB) AllReduce between the two cores sharing a batch
(replica groups [[0,1],[2,3],[4,5],[6,7]]).

Both softmaxes are folded into matmuls:
  - k-softmax over S: gmap = (exp(k)/colsum).T @ v
      == diag(1/colsum) @ (exp(k).T @ [v | 1]); the ones column produces
      colsum in the same PSUM accumulation, and the divide happens after
      the AllReduce on the 64x65-per-head global map.
  - q-softmax over HD: o = (exp(q) @ g) / rowsum(exp(q)); rowsum comes from
      a ones-masked matmul and the divide is fused into PSUM eviction.

exp() needs no max-subtraction here: k,q = x @ W.T with |entries| <~ 5, and
softmax is shift-invariant so the result matches the reference exactly up
to fp rounding.
"""

import sys

sys.path.insert(0, "/opt/trn_rl_repo")

import numpy as np
from contextlib import ExitStack

import concourse.bass as bass
import concourse.mybir as mybir
import concourse.tile as tile
from concourse.bass_utils import run_bass_kernel_spmd
from concourse.masks import make_identity

F32 = mybir.dt.float32
F32R = mybir.dt.float32r

B, S, DM, H, HD = 4, 4096, 1024, 16, 64
EPS = 1e-5
NCORES = 8
R = S // 2          # rows per core
P = 128             # partitions
NBLK = R // P       # 16 sequence blocks of 128 rows
NKT = DM // P       # 8 k-tiles over the contraction dim
NPAIR = DM // P     # 8 head-pairs (2 heads of 64 = 128 channels)
CHUNK = 512         # moving-operand width for the big projections
NCHUNK = R // CHUNK # 4
BPC = CHUNK // P    # 4 blocks per chunk


def _fix_multiwaits(nc):
    """This walrus build encodes at most one sync wait per instruction;
    split any multi-wait instruction into preceding same-engine NoOps."""
    for fn in nc.m.functions:
        for bb in fn.blocks:
            new_insts = []
            changed = False
            for ins in bb.instructions:
                si = ins.sync_info
                if si is not None and si.on_wait and len(si.on_wait) > 1:
                    waits = list(si.on_wait)
                    for i, w in enumerate(waits[:-1]):
                        new_insts.append(
                            mybir.InstNoOp(
                                name=f"{ins.name}-wsplit{i}",
                                engine=ins.engine,
                                sync_info=mybir.SyncInfo(on_wait=[w], on_update=[]),
                                bass_nofuse=True,
                            )
                        )
                    ins.sync_info = mybir.SyncInfo(
                        on_wait=[waits[-1]], on_update=list(si.on_update or [])
                    )
                    changed = True
                new_insts.append(ins)
            if changed:
                bb.instructions = new_insts


def _body(ctx, tc, io, flags):
    nc = tc.nc
    has_bq, has_bk, has_bv, has_gamma, has_beta = flags
    x_d, wqt_d, wkt_d, wvt_d, bq_d, bk_d, bv_d, gamma_d, beta_d, out_d = io

    const = ctx.enter_context(tc.tile_pool(name="const", bufs=1))
    wpool = ctx.enter_context(tc.tile_pool(name="w", bufs=1))
    xpool = ctx.enter_context(tc.tile_pool(name="x", bufs=3))
    x2pool = ctx.enter_context(tc.tile_pool(name="x2", bufs=6))
    xtpool = ctx.enter_context(tc.tile_pool(name="xt", bufs=3))
    xt2pool = ctx.enter_context(tc.tile_pool(name="xt2", bufs=1))
    kvpool = ctx.enter_context(tc.tile_pool(name="kv", bufs=2))
    eqpool = ctx.enter_context(tc.tile_pool(name="eq", bufs=1))
    opool = ctx.enter_context(tc.tile_pool(name="o", bufs=2))
    gpool = ctx.enter_context(tc.tile_pool(name="g", bufs=1))
    smpool = ctx.enter_context(tc.tile_pool(name="sm", bufs=3))
    dram = ctx.enter_context(tc.tile_pool(name="dram", bufs=1, space="DRAM"))

    ps_t = ctx.enter_context(tc.tile_pool(name="ps_t", bufs=2, space="PSUM"))
    ps_k = ctx.enter_context(tc.tile_pool(name="ps_k", bufs=2, space="PSUM"))
    ps_v = ctx.enter_context(tc.tile_pool(name="ps_v", bufs=2, space="PSUM"))
    ps_g = ctx.enter_context(tc.tile_pool(name="ps_g", bufs=2, space="PSUM"))
    # sweep 2 reuses the same budget: transposes share ps_t, q-proj shares
    # ps_k, o-matmuls share ps_v, qden shares ps_g (all sequential phases).

    # ---- constants / weights -------------------------------------------
    identity = const.tile([P, P], F32)
    make_identity(nc, identity[:])

    eps_t = const.tile([P, 1], F32)
    nc.vector.memset(eps_t[:], EPS)

    # ones-mask [128, 2]: col j selects the 64 partitions of head j in a pair
    hmask = const.tile([P, 2], F32)
    nc.vector.memset(hmask[:], 0.0)
    nc.vector.memset(hmask[0:64, 0:1], 1.0)
    nc.vector.memset(hmask[64:128, 1:2], 1.0)

    wq = [wpool.tile([P, DM], F32R, tag=f"wq{t}", name=f"wq{t}") for t in range(NKT)]
    wk = [wpool.tile([P, DM], F32R, tag=f"wk{t}", name=f"wk{t}") for t in range(NKT)]
    wv = [wpool.tile([P, DM], F32R, tag=f"wv{t}", name=f"wv{t}") for t in range(NKT)]
    # fp32r is a distinct bit format: DMA raw fp32 into staging, engines cast
    for dst, src_d in ((wk, wkt_d), (wv, wvt_d), (wq, wqt_d)):
        for t in range(NKT):
            stg = x2pool.tile([P, DM], F32, tag="x2b", name="wstg")
            nc.sync.dma_start(out=stg[:], in_=src_d[t * P:(t + 1) * P, :])
            nc.vector.tensor_copy(out=dst[t][:], in_=stg[:])

    bq_t = None
    if has_bq:
        bq_t = const.tile([P, NKT], F32)
        nc.sync.dma_start(out=bq_t[:], in_=bq_d.rearrange("(t p) -> p t", p=P))
    bk_bc = bv_bc = gamma_bc = beta_bc = None

    def _bcast(src_d):
        t = const.tile([P, DM], F32)
        src = bass.AP(tensor=src_d.tensor, offset=src_d.offset,
                      ap=[[0, P]] + list(src_d.ap))
        nc.sync.dma_start(out=t[:], in_=src)
        return t

    if has_bk:
        bk_bc = _bcast(bk_d)
    if has_bv:
        bv_bc = _bcast(bv_d)
    if has_gamma:
        gamma_bc = _bcast(gamma_d)
    if has_beta:
        beta_bc = _bcast(beta_d)

    # G accumulator [128, pair, 129]: cols 0..127 = 2-head block of kT@v
    # (only the two diagonal 64x64 blocks are meaningful), col 128 = colsum.
    gacc = gpool.tile([P, NPAIR, 129], F32)
    nc.vector.memset(gacc[:], 0.0)

    # ================= sweep 1: k/v projections + G ======================
    for b in range(NBLK):
        x_b = xpool.tile([P, DM], F32, tag="xb", name="xb")
        nc.sync.dma_start(out=x_b[:], in_=x_d[b * P:(b + 1) * P, :])

        # transpose x block -> xT (contraction layout)
        xt_b = [xtpool.tile([P, P], F32R, tag=f"xt{t}", name=f"xt{t}") for t in range(NKT)]
        for t in range(NKT):
            pt = ps_t.tile([P, P], F32, tag="pt", name="pt")
            nc.tensor.transpose(pt[:], x_b[:, t * P:(t + 1) * P], identity[:])
            nc.vector.tensor_copy(out=xt_b[t][:], in_=pt[:])

        # k and v projections for this block (weights stream as moving side)
        expk_b = kvpool.tile([P, DM], F32, tag="ek", name="ek")
        vext_b = kvpool.tile([P, NPAIR, 129], F32, tag="vx", name="vx")
        nc.gpsimd.memset(vext_b[:, :, 128:129], 1.0)
        for c in range(2):
            cs = slice(c * CHUNK, (c + 1) * CHUNK)
            pk = ps_k.tile([P, CHUNK], F32, tag="pk", name="pk")
            pv = ps_v.tile([P, CHUNK], F32, tag="pv", name="pv")
            for t in range(NKT):
                nc.tensor.matmul(pk[:], (xt_b[t][:]), (wk[t][:, cs]),
                                 start=(t == 0), stop=(t == NKT - 1))
                nc.tensor.matmul(pv[:], (xt_b[t][:]), (wv[t][:, cs]),
                                 start=(t == 0), stop=(t == NKT - 1))
            if has_bk:
                nc.vector.tensor_add(out=expk_b[:, cs], in0=pk[:], in1=bk_bc[:, cs])
                nc.scalar.activation(out=expk_b[:, cs], in_=expk_b[:, cs],
                                     func=mybir.ActivationFunctionType.Exp)
            else:
                nc.scalar.activation(out=expk_b[:, cs], in_=pk[:],
                                     func=mybir.ActivationFunctionType.Exp)
            vdst = vext_b[:, 4 * c:4 * (c + 1), 0:128]
            psrc = pv[:].rearrange("p (a b) -> p a b", a=4)
            if has_bv:
                nc.vector.tensor_add(
                    out=vdst, in0=psrc,
                    in1=bv_bc[:, cs].rearrange("p (a b) -> p a b", a=4))
            else:
                nc.vector.tensor_copy(out=vdst, in_=psrc)

        # G += expk_pair.T @ [v_pair | 1] for each head pair
        for p in range(NPAIR):
            pg = ps_g.tile([P, 129], F32, tag="pg", name="pg")
            nc.tensor.matmul(pg[:], (expk_b[:, p * P:(p + 1) * P]),
                             (vext_b[:, p, :]), start=True, stop=True)
            nc.vector.tensor_add(out=gacc[:, p, :], in0=gacc[:, p, :], in1=pg[:])

    # ================= AllReduce G within batch pairs ====================
    g_in = dram.tile([P, NPAIR, 129], F32)
    g_out = dram.tile([P, NPAIR, 129], F32)
    nc.gpsimd.dma_start(out=g_in[:], in_=gacc[:])
    nc.gpsimd.collective_compute(
        "AllReduce", mybir.AluOpType.add,
        replica_groups=[[0, 1], [2, 3], [4, 5], [6, 7]],
        ins=[g_in.opt()], outs=[g_out.opt()],
    )
    gall = gacc  # reuse the accumulator tile for the reduced result
    nc.gpsimd.dma_start(out=gall[:], in_=g_out[:])

    # g_bd[p] = blockdiag(g_2p, g_2p+1) scaled by 1/colsum (k-softmax denom)
    rcs = gpool.tile([P, NPAIR], F32)
    nc.vector.reciprocal(out=rcs[:], in_=gall[:, :, 128])
    # gs[j*64:(j+1)*64, p, :] = per-head g (64x64), scaled by 1/colsum
    gs = gpool.tile([P, NPAIR, HD], F32)
    for p in range(NPAIR):
        nc.vector.tensor_scalar_mul(out=gs[0:64, p, :],
                                    in0=gall[0:64, p, 0:64],
                                    scalar1=rcs[0:64, p:p + 1])
        nc.vector.tensor_scalar_mul(out=gs[64:128, p, :],
                                    in0=gall[64:128, p, 64:128],
                                    scalar1=rcs[64:128, p:p + 1])

    # ============ sweep 2: q projection + o + residual + LN ==============
    for c in range(NCHUNK):
        # re-DMA x (serves transpose input AND the residual add)
        x_blocks = []
        for j in range(BPC):
            b = c * BPC + j
            x_b = x2pool.tile([P, DM], F32, tag="x2b", name="x2b")
            nc.sync.dma_start(out=x_b[:], in_=x_d[b * P:(b + 1) * P, :])
            x_blocks.append(x_b)

        xt_c = [xt2pool.tile([P, BPC, P], F32R, tag=f"xt2_{t}", name=f"xt2_{t}") for t in range(NKT)]
        for j in range(BPC):
            for t in range(NKT):
                pt = ps_t.tile([P, P], F32, tag="pt", name="pt")
                nc.tensor.transpose(pt[:], x_blocks[j][:, t * P:(t + 1) * P],
                                    identity[:])
                nc.vector.tensor_copy(out=xt_c[t][:, j, :], in_=pt[:])

        # qT = Wq @ x.T for this chunk, evicted through Exp (channel-major:
        # the per-channel bias bq is a per-partition activation bias)
        eq_c = [eqpool.tile([P, CHUNK], F32, tag=f"eq{m}", name=f"eq{m}") for m in range(NKT)]
        for m in range(NKT):
            pq = ps_k.tile([P, CHUNK], F32, tag="pk", name="pk")
            for t in range(NKT):
                nc.tensor.matmul(
                    pq[:], (wq[t][:, m * P:(m + 1) * P]),
                    (xt_c[t][:].rearrange("p a b -> p (a b)")),
                    start=(t == 0), stop=(t == NKT - 1))
            if has_bq:
                nc.scalar.activation(out=eq_c[m][:], in_=pq[:],
                                     func=mybir.ActivationFunctionType.Exp,
                                     bias=bq_t[:, m:m + 1])
            else:
                nc.scalar.activation(out=eq_c[m][:], in_=pq[:],
                                     func=mybir.ActivationFunctionType.Exp)

        for j in range(BPC):
            b = c * BPC + j
            js = slice(j * P, (j + 1) * P)

            # q-softmax denominator: sum_d exp(q) via ones-masked matmul
            pqd = ps_g.tile([P, H], F32, tag="pg", name="pgq")
            for m in range(NKT):
                nc.tensor.matmul(pqd[:, 2 * m:2 * m + 2], (eq_c[m][:, js]),
                                 (hmask[:]), start=True, stop=True)
            rq = smpool.tile([P, H], F32, tag="rq", name="rq")
            nc.vector.reciprocal(out=rq[:], in_=pqd[:])

            o_b = opool.tile([P, DM], F32, tag="ob", name="ob")
            for p in range(NPAIR):
                for h2 in range(2):
                    h = 2 * p + h2
                    hs = slice(h2 * 64, h2 * 64 + 64)
                    po = ps_v.tile([P, HD], F32, tag="pv", name="po")
                    nc.tensor.matmul(po[:], eq_c[p][hs, js], gs[hs, p, :],
                                     start=True, stop=True)
                    nc.vector.tensor_scalar_mul(
                        out=o_b[:, h * HD:(h + 1) * HD],
                        in0=po[:],
                        scalar1=rq[:, h:h + 1])

            # y = x + o, then LayerNorm over DM
            nc.vector.tensor_add(out=o_b[:], in0=o_b[:], in1=x_blocks[j][:])
            stats = smpool.tile([P, 2, 6], F32, tag="st", name="st")
            yg = o_b[:].rearrange("p (a b) -> p a b", a=2)
            for i in range(2):
                nc.vector.bn_stats(out=stats[:, i, :], in_=yg[:, i, :])
            mv = smpool.tile([P, 2], F32, tag="mv", name="mv")
            nc.vector.bn_aggr(out=mv[:], in_=stats[:])
            nc.scalar.activation(out=mv[:, 1:2], in_=mv[:, 1:2],
                                 func=mybir.ActivationFunctionType.Sqrt,
                                 bias=eps_t[:])
            nc.vector.reciprocal(out=mv[:, 1:2], in_=mv[:, 1:2])
            nc.vector.tensor_scalar(out=o_b[:], in0=o_b[:],
                                    scalar1=mv[:, 0:1], scalar2=mv[:, 1:2],
                                    op0=mybir.AluOpType.subtract,
                                    op1=mybir.AluOpType.mult)
            if has_gamma:
                nc.vector.tensor_mul(out=o_b[:], in0=o_b[:], in1=gamma_bc[:])
            if has_beta:
                nc.vector.tensor_add(out=o_b[:], in0=o_b[:], in1=beta_bc[:])
            nc.sync.dma_start(out=out_d[b * P:(b + 1) * P, :], in_=o_b[:])


_PROGRAM_CACHE = {}


def _build_program(flags):
    if flags in _PROGRAM_CACHE:
        return _PROGRAM_CACHE[flags]
    nc = bass.Bass("TRN2", target_bir_lowering=False, debug=False,
                   num_devices=NCORES)
    x_d = nc.dram_tensor("x_shard", [R, DM], F32, kind="ExternalInput").ap()
    wqt_d = nc.dram_tensor("wq_t", [DM, DM], F32, kind="ExternalInput").ap()
    wkt_d = nc.dram_tensor("wk_t", [DM, DM], F32, kind="ExternalInput").ap()
    wvt_d = nc.dram_tensor("wv_t", [DM, DM], F32, kind="ExternalInput").ap()
    bq_d = nc.dram_tensor("bq", [DM], F32, kind="ExternalInput").ap()
    bk_d = nc.dram_tensor("bk", [DM], F32, kind="ExternalInput").ap()
    bv_d = nc.dram_tensor("bv", [DM], F32, kind="ExternalInput").ap()
    gamma_d = nc.dram_tensor("gamma", [DM], F32, kind="ExternalInput").ap()
    beta_d = nc.dram_tensor("beta", [DM], F32, kind="ExternalInput").ap()
    out_d = nc.dram_tensor("out_shard", [R, DM], F32, kind="ExternalOutput").ap()
    io = (x_d, wqt_d, wkt_d, wvt_d, bq_d, bk_d, bv_d, gamma_d, beta_d, out_d)
    with tile.TileContext(nc) as tc:
        with ExitStack() as ctx:
            _body(ctx, tc, io, flags)
    _fix_multiwaits(nc)
    _PROGRAM_CACHE[flags] = nc
    return nc


def kernel(x, mask, pad_mask, Wq, bq, Wk, bk, Wv, bv, gamma, beta):
    x = np.ascontiguousarray(np.asarray(x, dtype=np.float32))
    flags = (bool(np.any(bq)), bool(np.any(bk)), bool(np.any(bv)),
             bool(np.any(np.asarray(gamma) != 1.0)), bool(np.any(beta)))
    nc = _build_program(flags)

    wq_t = np.ascontiguousarray(np.asarray(Wq, dtype=np.float32).T)
    wk_t = np.ascontiguousarray(np.asarray(Wk, dtype=np.float32).T)
    wv_t = np.ascontiguousarray(np.asarray(Wv, dtype=np.float32).T)
    common = {
        "wq_t": wq_t, "wk_t": wk_t, "wv_t": wv_t,
        "bq": np.ascontiguousarray(bq, dtype=np.float32),
        "bk": np.ascontiguousarray(bk, dtype=np.float32),
        "bv": np.ascontiguousarray(bv, dtype=np.float32),
        "gamma": np.ascontiguousarray(gamma, dtype=np.float32),
        "beta": np.ascontiguousarray(beta, dtype=np.float32),
    }
    in_maps = []
    for c in range(NCORES):
        b, half = divmod(c, 2)
        shard = np.ascontiguousarray(x[b, half * R:(half + 1) * R, :])
        in_maps.append({"x_shard": shard, **common})

    res = run_bass_kernel_spmd(nc, in_maps, list(range(NCORES)))

    out = np.empty((B, S, DM), dtype=np.float32)
    for c in range(NCORES):
        b, half = divmod(c, 2)
        out[b, half * R:(half + 1) * R, :] = res.results[c]["out_shard"]
    return out


if __name__ == "__main__":
    rng = np.random.default_rng(0)
    demo = {
        "x": rng.standard_normal((B, S, DM), dtype=np.float32),
        "mask": np.zeros((S, S), bool),
        "pad_mask": np.zeros((B, S), bool),
        "Wq": rng.uniform(-0.03, 0.03, (DM, DM)).astype(np.float32),
        "bq": np.zeros(DM, np.float32),
        "Wk": rng.uniform(-0.03, 0.03, (DM, DM)).astype(np.float32),
        "bk": np.zeros(DM, np.float32),
        "Wv": rng.uniform(-0.03, 0.03, (DM, DM)).astype(np.float32),
        "bv": np.zeros(DM, np.float32),
        "gamma": np.ones(DM, np.float32),
        "beta": np.zeros(DM, np.float32),
    }
    out = kernel(**demo)
    print("out", out.shape, out.dtype, float(np.abs(out).max()))


# revision 18
# speedup vs baseline: 1.5080x; 1.5080x over previous
"""Trainium2 Bass kernel for nn_MultiHeadAttention_Linear_11312943857747.

Math (B=4, S=4096, DM=1024, H=16, HD=64):
    q = softmax(x @ Wq.T + bq) over head_dim
    k = softmax(x @ Wk.T + bk) over seq_len
    v = x @ Wv.T + bv
    gmap[b,h] = k[b,h].T @ v[b,h]            (HD x HD per head)
    o[b,h]    = q[b,h] @ gmap[b,h]
    out = LayerNorm(x + o) * gamma + beta

Sharding: 8 cores = 4 batches x 2 sequence-halves. Each core projects its
2048 rows; the per-head kT@v reduction over the full sequence is completed
with a tiny (132KB) AllReduce between the two cores sharing a batch
(replica groups [[0,1],[2,3],[4,5],[6,7]]).

Both softmaxes are folded into matmuls:
  - k-softmax over S: gmap = (exp(k)/colsum).T @ v
      == diag(1/colsum) @ (exp(k).T @ [v | 1]); the ones column produces
      colsum in the same PSUM accumulation, and the divide happens after
      the AllReduce on the 64x65-per-head global map.
  - q-softmax over HD: o = (exp(q) @ g) / rowsum(exp(q)); rowsum comes from
      a ones-masked matmul and the divide is fused into PSUM eviction.

exp() needs no max-subtraction here: k,q = x @ W.T with |entries| <~ 5, and
softmax is shift-invariant so the result matches the reference exactly up
to fp rounding.

Matmul operands are cast to bf16 on-chip (PSUM accumulation stays fp32):
fp32/fp32r matmuls run as two half-rate passes on TRN2, bf16 single-pass.
The host supplies x in both layouts (x and x.T) so no PE transposes are
needed; the contraction layout is required by the TensorE (stationary and
moving operands both index the contraction dim on partitions).
"""

import sys

sys.path.insert(0, "/opt/trn_rl_repo")

import numpy as np
from contextlib import ExitStack

import concourse.bass as bass
import concourse.mybir as mybir
import concourse.tile as tile
from concourse.bass_utils import run_bass_kernel_spmd

F32 = mybir.dt.float32
BF16 = mybir.dt.bfloat16

B, S, DM, H, HD = 4, 4096, 1024, 16, 64
EPS = 1e-5
NCORES = 8
R = S // 2          # rows per core
P = 128             # partitions
NBLK = R // P       # 16 sequence blocks of 128 rows
NKT = DM // P       # 8 k-tiles over the contraction dim
NPAIR = DM // P     # 8 head-pairs (2 heads of 64 = 128 channels)
CHUNK = 512         # moving-operand width for the big projections
NCHUNK = R // CHUNK # 4
BPC = CHUNK // P    # 4 blocks per chunk


def _fix_multiwaits(nc):
    """This walrus build encodes at most one sync wait per instruction;
    split any multi-wait instruction into preceding same-engine NoOps."""
    for fn in nc.m.functions:
        for bb in fn.blocks:
            new_insts = []
            changed = False
            for ins in bb.instructions:
                si = ins.sync_info
                if si is not None and si.on_wait and len(si.on_wait) > 1:
                    waits = list(si.on_wait)
                    for i, w in enumerate(waits[:-1]):
                        new_insts.append(
                            mybir.InstNoOp(
                                name=f"{ins.name}-wsplit{i}",
                                engine=ins.engine,
                                sync_info=mybir.SyncInfo(on_wait=[w], on_update=[]),
                                bass_nofuse=True,
                            )
                        )
                    ins.sync_info = mybir.SyncInfo(
                        on_wait=[waits[-1]], on_update=list(si.on_update or [])
                    )
                    changed = True
                new_insts.append(ins)
            if changed:
                bb.instructions = new_insts


def _body(ctx, tc, io, flags):
    import os
    bisect = os.environ.get("KBISECT", "")
    nc = tc.nc
    has_bq, has_bk, has_bv, has_gamma, has_beta = flags
    (x_d, xt_d, wqt_d, wkt_d, wvt_d, bq_d, bk_d, bv_d, gamma_d, beta_d,
     out_d) = io

    const = ctx.enter_context(tc.tile_pool(name="const", bufs=1))
    wpool = ctx.enter_context(tc.tile_pool(name="w", bufs=1))
    stgpool = ctx.enter_context(tc.tile_pool(name="stg", bufs=6))
    xtpool = ctx.enter_context(tc.tile_pool(name="xt", bufs=1))
    x2pool = ctx.enter_context(tc.tile_pool(name="x2", bufs=4))
    kvpool = ctx.enter_context(tc.tile_pool(name="kv", bufs=3))
    eqpool = ctx.enter_context(tc.tile_pool(name="eq", bufs=1))
    opool = ctx.enter_context(tc.tile_pool(name="o", bufs=3))
    gpool = ctx.enter_context(tc.tile_pool(name="g", bufs=1))
    smpool = ctx.enter_context(tc.tile_pool(name="sm", bufs=3))
    dram = ctx.enter_context(tc.tile_pool(name="dram", bufs=1, space="DRAM"))

    ps_k = ctx.enter_context(tc.tile_pool(name="ps_k", bufs=2, space="PSUM"))
    ps_v = ctx.enter_context(tc.tile_pool(name="ps_v", bufs=2, space="PSUM"))
    ps_g = ctx.enter_context(tc.tile_pool(name="ps_g", bufs=2, space="PSUM"))
    ps_q = ctx.enter_context(tc.tile_pool(name="ps_q", bufs=2, space="PSUM"))
    # sweep 2 reuses the budget: qden shares ps_g, o-matmuls share ps_v.

    # ---- constants -----------------------------------------------------
    eps_t = const.tile([P, 1], F32)
    nc.vector.memset(eps_t[:], EPS)

    # ones-mask [128, 2]: col j selects the 64 partitions of head j in a pair
    hmask = const.tile([P, 2], BF16)
    nc.vector.memset(hmask[:], 0.0)
    nc.vector.memset(hmask[0:64, 0:1], 1.0)
    nc.vector.memset(hmask[64:128, 1:2], 1.0)

    # ---- weights: DMA fp32 staging -> bf16 tiles -----------------------
    wq = [wpool.tile([P, DM], BF16, tag=f"wq{t}", name=f"wq{t}") for t in range(NKT)]
    wk = [wpool.tile([P, DM], BF16, tag=f"wk{t}", name=f"wk{t}") for t in range(NKT)]
    wv = [wpool.tile([P, DM], BF16, tag=f"wv{t}", name=f"wv{t}") for t in range(NKT)]
    for dst, src_d in ((wk, wkt_d), (wv, wvt_d), (wq, wqt_d)):
        for t in range(NKT):
            stg = stgpool.tile([P, DM], F32, tag="stg", name="wstg")
            nc.sync.dma_start(out=stg[:], in_=src_d[t * P:(t + 1) * P, :])
            nc.vector.tensor_copy(out=dst[t][:], in_=stg[:])

    # ---- x.T: DMA fp32 staging -> bf16 tiles (gpsimd does the cast) ----
    # xt[t][c] is [128, 512] bf16: contraction k-tile t, seq chunk c.
    xt = [[xtpool.tile([P, CHUNK], BF16, tag=f"xt{t}_{c}", name=f"xt{t}_{c}")
           for c in range(NCHUNK)] for t in range(NKT)]
    for c in range(NCHUNK):
        for t in range(NKT):
            stg = stgpool.tile([P, CHUNK], F32, tag="stg", name="xtstg")
            nc.sync.dma_start(
                out=stg[:],
                in_=xt_d[t * P:(t + 1) * P, c * CHUNK:(c + 1) * CHUNK])
            nc.vector.tensor_copy(out=xt[t][c][:], in_=stg[:])

    bq_t = None
    if has_bq:
        bq_t = const.tile([P, NKT], F32)
        nc.sync.dma_start(out=bq_t[:], in_=bq_d.rearrange("(t p) -> p t", p=P))
    bk_bc = bv_bc = gamma_bc = beta_bc = None

    def _bcast(src_d):
        t = const.tile([P, DM], F32, name=f"bc_{src_d.tensor.name}")
        src = bass.AP(tensor=src_d.tensor, offset=src_d.offset,
                      ap=[[0, P]] + list(src_d.ap))
        nc.sync.dma_start(out=t[:], in_=src)
        return t

    if has_bk:
        bk_bc = _bcast(bk_d)
    if has_bv:
        bv_bc = _bcast(bv_d)
    if has_gamma:
        gamma_bc = _bcast(gamma_d)
    if has_beta:
        beta_bc = _bcast(beta_d)

    # G accumulator [128, pair, 129]: cols 0..127 = 2-head block of kT@v
    # (only the two diagonal 64x64 blocks are meaningful), col 128 = colsum.
    gacc = gpool.tile([P, NPAIR, 130], F32)
    nc.vector.memset(gacc[:], 0.0)

    # eq[c][m]: exp(q).T for chunk c, channel tile m — bf16, all resident
    eq = [[eqpool.tile([P, CHUNK], BF16, tag=f"eq{c}_{m}", name=f"eq{c}_{m}")
           for m in range(NKT)] for c in range(NCHUNK)]

    # ============ sweep 1: k/v/q projections + G accumulation ===========
    for b in range(NBLK):
        c, j = divmod(b, BPC)
        js = slice(j * P, (j + 1) * P)

        expk_b = kvpool.tile([P, DM], BF16, tag="ek", name="ek")
        vext_b = kvpool.tile([P, NPAIR, 130], BF16, tag="vx", name="vx")
        nc.vector.memset(vext_b[:, :, 128:130], 1.0)
        for cc in range(2):
            cs = slice(cc * CHUNK, (cc + 1) * CHUNK)
            pk = ps_k.tile([P, CHUNK], F32, tag="pk", name="pk")
            pv = ps_v.tile([P, CHUNK], F32, tag="pv", name="pv")
            for t in range(NKT):
                lhsT = xt[t][c][:, js]
                nc.tensor.matmul(pk[:], lhsT, wk[t][:, cs],
                                 start=(t == 0), stop=(t == NKT - 1))
                nc.tensor.matmul(pv[:], lhsT, wv[t][:, cs],
                                 start=(t == 0), stop=(t == NKT - 1))
            if has_bk:
                nc.vector.tensor_add(out=expk_b[:, cs], in0=pk[:], in1=bk_bc[:, cs])
                nc.scalar.activation(out=expk_b[:, cs], in_=expk_b[:, cs],
                                     func=mybir.ActivationFunctionType.Exp)
            else:
                nc.scalar.activation(out=expk_b[:, cs], in_=pk[:],
                                     func=mybir.ActivationFunctionType.Exp)
            vdst = vext_b[:, 4 * cc:4 * (cc + 1), 0:128]
            psrc = pv[:].rearrange("p (a b) -> p a b", a=4)
            if has_bv:
                nc.vector.tensor_add(
                    out=vdst, in0=psrc,
                    in1=bv_bc[:, cs].rearrange("p (a b) -> p a b", a=4))
            else:
                nc.vector.tensor_copy(out=vdst, in_=psrc)

        # G += expk_pair.T @ [v_pair | 1]; two pairs share one PSUM bank
        for i in range(NPAIR // 2):
            pg = ps_g.tile([P, 2, 130], F32, tag="pg", name="pg")
            for u in range(2):
                p = 2 * i + u
                nc.tensor.matmul(pg[:, u, :], expk_b[:, p * P:(p + 1) * P],
                                 vext_b[:, p, :], start=True, stop=True)
            nc.vector.tensor_add(out=gacc[:, 2 * i:2 * i + 2, :],
                                 in0=gacc[:, 2 * i:2 * i + 2, :], in1=pg[:])

        # after each chunk's 4 blocks: q-projection for that chunk
        if j == BPC - 1:
            for m in range(NKT):
                pq = ps_q.tile([P, CHUNK], F32, tag="pq", name="pq")
                for t in range(NKT):
                    nc.tensor.matmul(pq[:], wq[t][:, m * P:(m + 1) * P],
                                     xt[t][c][:], start=(t == 0),
                                     stop=(t == NKT - 1))
                if has_bq:
                    nc.scalar.activation(out=eq[c][m][:], in_=pq[:],
                                         func=mybir.ActivationFunctionType.Exp,
                                         bias=bq_t[:, m:m + 1])
                else:
                    nc.scalar.activation(out=eq[c][m][:], in_=pq[:],
                                         func=mybir.ActivationFunctionType.Exp)

    # ================= AllReduce G within batch pairs ====================
    g_in = dram.tile([P, NPAIR, 130], F32)
    g_out = dram.tile([P, NPAIR, 130], F32)
    nc.gpsimd.dma_start(out=g_in[:], in_=gacc[:])
    nc.gpsimd.collective_compute(
        "AllReduce", mybir.AluOpType.add,
        replica_groups=[[0, 1], [2, 3], [4, 5], [6, 7]],
        ins=[g_in.opt()], outs=[g_out.opt()],
    )
    gall = gacc  # reuse the accumulator tile for the reduced result
    nc.gpsimd.dma_start(out=gall[:], in_=g_out[:])

    # gs[j*64:(j+1)*64, p, :] = per-head g (64x64), scaled by 1/colsum
    rcs = gpool.tile([P, NPAIR], F32)
    nc.vector.reciprocal(out=rcs[:], in_=gall[:, :, 128])
    # block-diagonal per-pair g (off-diagonal cross-head blocks zeroed) so
    # each pair's o needs ONE full-base matmul: two matmuls into the same
    # PSUM bank with mismatched tile_position row bases hang the device.
    g_bd = gpool.tile([P, NPAIR, P], BF16)
    nc.vector.memset(g_bd[:], 0.0)
    for p in range(NPAIR):
        nc.vector.tensor_scalar_mul(out=g_bd[0:64, p, 0:64],
                                    in0=gall[0:64, p, 0:64],
                                    scalar1=rcs[0:64, p:p + 1])
        nc.vector.tensor_scalar_mul(out=g_bd[64:128, p, 64:128],
                                    in0=gall[64:128, p, 64:128],
                                    scalar1=rcs[64:128, p:p + 1])

    # ============ sweep 2: o = softmax(q) @ g, residual, LN ==============
    for b in range(NBLK):
        if bisect == "s1":
            x_b = x2pool.tile([P, DM], F32, tag="x2b", name="x2b")
            nc.sync.dma_start(out=x_b[:], in_=x_d[b * P:(b + 1) * P, :])
            nc.sync.dma_start(out=out_d[b * P:(b + 1) * P, :], in_=x_b[:])
            continue
        c, j = divmod(b, BPC)
        js = slice(j * P, (j + 1) * P)

        x_b = x2pool.tile([P, DM], F32, tag="x2b", name="x2b")
        nc.sync.dma_start(out=x_b[:], in_=x_d[b * P:(b + 1) * P, :])

        # q-softmax denominator: sum_d exp(q) via ones-masked matmul
        pqd = ps_g.tile([P, H], F32, tag="pg", name="pqd")
        for m in range(NKT):
            nc.tensor.matmul(pqd[:, 2 * m:2 * m + 2], eq[c][m][:, js],
                             hmask[:], start=True, stop=True)
        rq = smpool.tile([P, H], F32, tag="rq", name="rq")
        nc.vector.reciprocal(out=rq[:], in_=pqd[:])

        if bisect == "s2qden":
            nc.sync.dma_start(out=out_d[b * P:(b + 1) * P, :], in_=x_b[:])
            continue
        # o matmuls per pair; divide by the q-softmax denominator during
        # eviction (inner step-0 AP broadcasts 1/qden over head_dim)
        o_b = opool.tile([P, DM], F32, tag="ob", name="ob")
        for p in range(NPAIR):
            po = ps_v.tile([P, 2, HD], F32, tag="pv", name="po")
            nc.tensor.matmul(po[:].rearrange("p a b -> p (a b)"),
                             eq[c][p][:, js], g_bd[:, p, :],
                             start=True, stop=True)
            import os as _os
            if _os.environ.get("KBISECT") == "s2o_copy":
                nc.vector.tensor_copy(
                    out=o_b[:, p * P:(p + 1) * P].rearrange(
                        "p (h d) -> p h d", h=2),
                    in_=po[:])
            else:
                rqs = rq[:, 2 * p:2 * p + 2]
                rq_bc = bass.AP(tensor=rqs.tensor, offset=rqs.offset,
                                ap=list(rqs.ap) + [[0, HD]])
                nc.vector.tensor_mul(
                    out=o_b[:, p * P:(p + 1) * P].rearrange(
                        "p (h d) -> p h d", h=2),
                    in0=po[:], in1=rq_bc)

        if bisect in ("s2o", "s2o_copy"):
            nc.sync.dma_start(out=out_d[b * P:(b + 1) * P, :], in_=o_b[:])
            continue
        # y = x + o, then LayerNorm over DM
        nc.vector.tensor_add(out=o_b[:], in0=o_b[:], in1=x_b[:])
        stats = smpool.tile([P, 2, 6], F32, tag="st", name="st")
        yg = o_b[:].rearrange("p (a b) -> p a b", a=2)
        for i in range(2):
            nc.vector.bn_stats(out=stats[:, i, :], in_=yg[:, i, :])
        mv = smpool.tile([P, 2], F32, tag="mv", name="mv")
        nc.vector.bn_aggr(out=mv[:], in_=stats[:])
        nc.scalar.activation(out=mv[:, 1:2], in_=mv[:, 1:2],
                             func=mybir.ActivationFunctionType.Sqrt,
                             bias=eps_t[:])
        nc.vector.reciprocal(out=mv[:, 1:2], in_=mv[:, 1:2])
        nc.vector.tensor_scalar(out=o_b[:], in0=o_b[:],
                                scalar1=mv[:, 0:1], scalar2=mv[:, 1:2],
                                op0=mybir.AluOpType.subtract,
                                op1=mybir.AluOpType.mult)
        if has_gamma:
            nc.vector.tensor_mul(out=o_b[:], in0=o_b[:], in1=gamma_bc[:])
        if has_beta:
            nc.vector.tensor_add(out=o_b[:], in0=o_b[:], in1=beta_bc[:])
        nc.sync.dma_start(out=out_d[b * P:(b + 1) * P, :], in_=o_b[:])


_PROGRAM_CACHE = {}


def _build_program(flags):
    if flags in _PROGRAM_CACHE:
        return _PROGRAM_CACHE[flags]
    nc = bass.Bass("TRN2", target_bir_lowering=False, debug=False,
                   num_devices=NCORES)
    x_d = nc.dram_tensor("x_shard", [R, DM], F32, kind="ExternalInput").ap()
    xt_d = nc.dram_tensor("xt_shard", [DM, R], F32, kind="ExternalInput").ap()
    wqt_d = nc.dram_tensor("wq_t", [DM, DM], F32, kind="ExternalInput").ap()
    wkt_d = nc.dram_tensor("wk_t", [DM, DM], F32, kind="ExternalInput").ap()
    wvt_d = nc.dram_tensor("wv_t", [DM, DM], F32, kind="ExternalInput").ap()
    bq_d = nc.dram_tensor("bq", [DM], F32, kind="ExternalInput").ap()
    bk_d = nc.dram_tensor("bk", [DM], F32, kind="ExternalInput").ap()
    bv_d = nc.dram_tensor("bv", [DM], F32, kind="ExternalInput").ap()
    gamma_d = nc.dram_tensor("gamma", [DM], F32, kind="ExternalInput").ap()
    beta_d = nc.dram_tensor("beta", [DM], F32, kind="ExternalInput").ap()
    out_d = nc.dram_tensor("out_shard", [R, DM], F32, kind="ExternalOutput").ap()
    io = (x_d, xt_d, wqt_d, wkt_d, wvt_d, bq_d, bk_d, bv_d, gamma_d, beta_d,
          out_d)
    with tile.TileContext(nc) as tc:
        with ExitStack() as ctx:
            _body(ctx, tc, io, flags)
    _fix_multiwaits(nc)
    _PROGRAM_CACHE[flags] = nc
    return nc


def kernel(x, mask, pad_mask, Wq, bq, Wk, bk, Wv, bv, gamma, beta):
    x = np.ascontiguousarray(np.asarray(x, dtype=np.float32))
    flags = (bool(np.any(bq)), bool(np.any(bk)), bool(np.any(bv)),
             bool(np.any(np.asarray(gamma) != 1.0)), bool(np.any(beta)))
    nc = _build_program(flags)

    common = {
        "wq_t": np.ascontiguousarray(np.asarray(Wq, dtype=np.float32).T),
        "wk_t": np.ascontiguousarray(np.asarray(Wk, dtype=np.float32).T),
        "wv_t": np.ascontiguousarray(np.asarray(Wv, dtype=np.float32).T),
        "bq": np.ascontiguousarray(bq, dtype=np.float32),
        "bk": np.ascontiguousarray(bk, dtype=np.float32),
        "bv": np.ascontiguousarray(bv, dtype=np.float32),
        "gamma": np.ascontiguousarray(gamma, dtype=np.float32),
        "beta": np.ascontiguousarray(beta, dtype=np.float32),
    }
    in_maps = []
    for c in range(NCORES):
        b, half = divmod(c, 2)
        shard = np.ascontiguousarray(x[b, half * R:(half + 1) * R, :])
        in_maps.append({"x_shard": shard,
                        "xt_shard": np.ascontiguousarray(shard.T),
                        **common})

    res = run_bass_kernel_spmd(nc, in_maps, list(range(NCORES)))

    out = np.empty((B, S, DM), dtype=np.float32)
    for c in range(NCORES):
        b, half = divmod(c, 2)
        out[b, half * R:(half + 1) * R, :] = res.results[c]["out_shard"]
    return out


if __name__ == "__main__":
    rng = np.random.default_rng(0)
    demo = {
        "x": rng.standard_normal((B, S, DM), dtype=np.float32),
        "mask": np.zeros((S, S), bool),
        "pad_mask": np.zeros((B, S), bool),
        "Wq": rng.uniform(-0.03, 0.03, (DM, DM)).astype(np.float32),
        "bq": np.zeros(DM, np.float32),
        "Wk": rng.uniform(-0.03, 0.03, (DM, DM)).astype(np.float32),
        "bk": np.zeros(DM, np.float32),
        "Wv": rng.uniform(-0.03, 0.03, (DM, DM)).astype(np.float32),
        "bv": np.zeros(DM, np.float32),
        "gamma": np.ones(DM, np.float32),
        "beta": np.zeros(DM, np.float32),
    }
    out = kernel(**demo)
    print("out", out.shape, out.dtype, float(np.abs(out).max()))


# revision 24
# speedup vs baseline: 1.6599x; 1.1007x over previous
"""Trainium2 Bass kernel for nn_MultiHeadAttention_Linear_11312943857747.

Math (B=4, S=4096, DM=1024, H=16, HD=64):
    q = softmax(x @ Wq.T + bq) over head_dim
    k = softmax(x @ Wk.T + bk) over seq_len
    v = x @ Wv.T + bv
    gmap[b,h] = k[b,h].T @ v[b,h]            (HD x HD per head)
    o[b,h]    = q[b,h] @ gmap[b,h]
    out = LayerNorm(x + o) * gamma + beta

Sharding: 8 cores = 4 batches x 2 sequence-halves. Each core projects its
2048 rows; the per-head kT@v reduction over the full sequence is completed
with a tiny (132KB) AllReduce between the two cores sharing a batch
(replica groups [[0,1],[2,3],[4,5],[6,7]]).

Both softmaxes are folded into matmuls:
  - k-softmax over S: gmap = (exp(k)/colsum).T @ v
      == diag(1/colsum) @ (exp(k).T @ [v | 1]); the ones column produces
      colsum in the same PSUM accumulation, and the divide happens after
      the AllReduce on the 64x65-per-head global map.
  - q-softmax over HD: o = (exp(q) @ g) / rowsum(exp(q)); rowsum comes from
      a ones-masked matmul and the divide is fused into PSUM eviction.

exp() needs no max-subtraction here: k,q = x @ W.T with |entries| <~ 5, and
softmax is shift-invariant so the result matches the reference exactly up
to fp rounding.

Matmul operands are cast to bf16 on-chip (PSUM accumulation stays fp32):
fp32/fp32r matmuls run as two half-rate passes on TRN2, bf16 single-pass.
The host supplies x in both layouts (x and x.T) so no PE transposes are
needed; the contraction layout is required by the TensorE (stationary and
moving operands both index the contraction dim on partitions).
"""

import sys

sys.path.insert(0, "/opt/trn_rl_repo")

import numpy as np
from contextlib import ExitStack

import concourse.bass as bass
import concourse.mybir as mybir
import concourse.tile as tile
from concourse.bass_utils import run_bass_kernel_spmd

F32 = mybir.dt.float32
BF16 = mybir.dt.bfloat16

B, S, DM, H, HD = 4, 4096, 1024, 16, 64
EPS = 1e-5
NCORES = 8
R = S // 2          # rows per core
P = 128             # partitions
NBLK = R // P       # 16 sequence blocks of 128 rows
NKT = DM // P       # 8 k-tiles over the contraction dim
NPAIR = DM // P     # 8 head-pairs (2 heads of 64 = 128 channels)
CHUNK = 512         # moving-operand width for the big projections
NCHUNK = R // CHUNK # 4
BPC = CHUNK // P    # 4 blocks per chunk


def _fix_multiwaits(nc):
    """This walrus build encodes at most one sync wait per instruction;
    split any multi-wait instruction into preceding same-engine NoOps."""
    for fn in nc.m.functions:
        for bb in fn.blocks:
            new_insts = []
            changed = False
            for ins in bb.instructions:
                si = ins.sync_info
                if si is not None and si.on_wait and len(si.on_wait) > 1:
                    waits = list(si.on_wait)
                    for i, w in enumerate(waits[:-1]):
                        new_insts.append(
                            mybir.InstNoOp(
                                name=f"{ins.name}-wsplit{i}",
                                engine=ins.engine,
                                sync_info=mybir.SyncInfo(on_wait=[w], on_update=[]),
                                bass_nofuse=True,
                            )
                        )
                    ins.sync_info = mybir.SyncInfo(
                        on_wait=[waits[-1]], on_update=list(si.on_update or [])
                    )
                    changed = True
                new_insts.append(ins)
            if changed:
                bb.instructions = new_insts


def _body(ctx, tc, io, flags):
    nc = tc.nc
    has_bq, has_bk, has_bv, has_gamma, has_beta = flags
    (x_d, xt_d, wqt_d, wkt_d, wvt_d, bq_d, bk_d, bv_d, gamma_d, beta_d,
     out_d) = io

    const = ctx.enter_context(tc.tile_pool(name="const", bufs=1))
    wpool = ctx.enter_context(tc.tile_pool(name="w", bufs=1))
    stgpool = ctx.enter_context(tc.tile_pool(name="stg", bufs=6))
    xtpool = ctx.enter_context(tc.tile_pool(name="xt", bufs=1))
    x2pool = ctx.enter_context(tc.tile_pool(name="x2", bufs=8))
    kvpool = ctx.enter_context(tc.tile_pool(name="kv", bufs=3))
    eqpool = ctx.enter_context(tc.tile_pool(name="eq", bufs=1))
    opool = ctx.enter_context(tc.tile_pool(name="o", bufs=3))
    gpool = ctx.enter_context(tc.tile_pool(name="g", bufs=1))
    smpool = ctx.enter_context(tc.tile_pool(name="sm", bufs=3))
    dram = ctx.enter_context(tc.tile_pool(name="dram", bufs=1, space="DRAM"))

    ps_k = ctx.enter_context(tc.tile_pool(name="ps_k", bufs=2, space="PSUM"))
    ps_v = ctx.enter_context(tc.tile_pool(name="ps_v", bufs=2, space="PSUM"))
    ps_g = ctx.enter_context(tc.tile_pool(name="ps_g", bufs=2, space="PSUM"))
    ps_q = ctx.enter_context(tc.tile_pool(name="ps_q", bufs=2, space="PSUM"))
    # sweep 2 reuses the budget: qden shares ps_g, o-matmuls share ps_v.

    # ---- constants -----------------------------------------------------
    eps_t = const.tile([P, 1], F32)
    nc.vector.memset(eps_t[:], EPS)

    # ones-mask [128, 2]: col j selects the 64 partitions of head j in a pair
    hmask = const.tile([P, 2], BF16)
    nc.vector.memset(hmask[:], 0.0)
    nc.vector.memset(hmask[0:64, 0:1], 1.0)
    nc.vector.memset(hmask[64:128, 1:2], 1.0)

    # ---- weights: DMA fp32 staging -> bf16 tiles -----------------------
    wq = [wpool.tile([P, DM], BF16, tag=f"wq{t}", name=f"wq{t}") for t in range(NKT)]
    wk = [wpool.tile([P, DM], BF16, tag=f"wk{t}", name=f"wk{t}") for t in range(NKT)]
    wv = [wpool.tile([P, DM], BF16, tag=f"wv{t}", name=f"wv{t}") for t in range(NKT)]
    for dst, src_d in ((wk, wkt_d), (wv, wvt_d), (wq, wqt_d)):
        for t in range(NKT):
            stg = stgpool.tile([P, DM], F32, tag="stg", name="wstg")
            nc.sync.dma_start(out=stg[:], in_=src_d[t * P:(t + 1) * P, :])
            nc.vector.tensor_copy(out=dst[t][:], in_=stg[:])

    # ---- x.T: DMA fp32 staging -> bf16 tiles (gpsimd does the cast) ----
    # xt[t][c] is [128, 512] bf16: contraction k-tile t, seq chunk c.
    xt = [[xtpool.tile([P, CHUNK], BF16, tag=f"xt{t}_{c}", name=f"xt{t}_{c}")
           for c in range(NCHUNK)] for t in range(NKT)]
    for c in range(NCHUNK):
        for t in range(NKT):
            stg = stgpool.tile([P, CHUNK], F32, tag="stg", name="xtstg")
            nc.sync.dma_start(
                out=stg[:],
                in_=xt_d[t * P:(t + 1) * P, c * CHUNK:(c + 1) * CHUNK])
            nc.vector.tensor_copy(out=xt[t][c][:], in_=stg[:])

    bq_t = None
    if has_bq:
        bq_t = const.tile([P, NKT], F32)
        nc.sync.dma_start(out=bq_t[:], in_=bq_d.rearrange("(t p) -> p t", p=P))
    bk_bc = bv_bc = gamma_bc = beta_bc = None

    def _bcast(src_d):
        t = const.tile([P, DM], F32, name=f"bc_{src_d.tensor.name}")
        src = bass.AP(tensor=src_d.tensor, offset=src_d.offset,
                      ap=[[0, P]] + list(src_d.ap))
        nc.sync.dma_start(out=t[:], in_=src)
        return t

    if has_bk:
        bk_bc = _bcast(bk_d)
    if has_bv:
        bv_bc = _bcast(bv_d)
    if has_gamma:
        gamma_bc = _bcast(gamma_d)
    if has_beta:
        beta_bc = _bcast(beta_d)

    # G accumulator [128, pair, 129]: cols 0..127 = 2-head block of kT@v
    # (only the two diagonal 64x64 blocks are meaningful), col 128 = colsum.
    gacc = gpool.tile([P, NPAIR, 130], F32)
    nc.vector.memset(gacc[:], 0.0)

    # eq[c][m]: exp(q).T for chunk c, channel tile m — bf16, all resident
    eq = [[eqpool.tile([P, CHUNK], BF16, tag=f"eq{c}_{m}", name=f"eq{c}_{m}")
           for m in range(NKT)] for c in range(NCHUNK)]

    # ============ sweep 1: k/v/q projections + G accumulation ===========
    for b in range(NBLK):
        c, j = divmod(b, BPC)
        js = slice(j * P, (j + 1) * P)

        expk_b = kvpool.tile([P, DM], BF16, tag="ek", name="ek")
        vext_b = kvpool.tile([P, NPAIR, 130], BF16, tag="vx", name="vx")
        nc.vector.memset(vext_b[:, :, 128:130], 1.0)
        for cc in range(2):
            cs = slice(cc * CHUNK, (cc + 1) * CHUNK)
            pk = ps_k.tile([P, CHUNK], F32, tag="pk", name="pk")
            pv = ps_v.tile([P, CHUNK], F32, tag="pv", name="pv")
            for t in range(NKT):
                lhsT = xt[t][c][:, js]
                nc.tensor.matmul(pk[:], lhsT, wk[t][:, cs],
                                 start=(t == 0), stop=(t == NKT - 1))
                nc.tensor.matmul(pv[:], lhsT, wv[t][:, cs],
                                 start=(t == 0), stop=(t == NKT - 1))
            if has_bk:
                nc.vector.tensor_add(out=expk_b[:, cs], in0=pk[:], in1=bk_bc[:, cs])
                nc.scalar.activation(out=expk_b[:, cs], in_=expk_b[:, cs],
                                     func=mybir.ActivationFunctionType.Exp)
            else:
                nc.scalar.activation(out=expk_b[:, cs], in_=pk[:],
                                     func=mybir.ActivationFunctionType.Exp)
            vdst = vext_b[:, 4 * cc:4 * (cc + 1), 0:128]
            psrc = pv[:].rearrange("p (a b) -> p a b", a=4)
            if has_bv:
                nc.vector.tensor_add(
                    out=vdst, in0=psrc,
                    in1=bv_bc[:, cs].rearrange("p (a b) -> p a b", a=4))
            else:
                nc.vector.tensor_copy(out=vdst, in_=psrc)

        # G += expk_pair.T @ [v_pair | 1]; two pairs share one PSUM bank
        for i in range(NPAIR // 2):
            pg = ps_g.tile([P, 2, 130], F32, tag="pg", name="pg")
            for u in range(2):
                p = 2 * i + u
                nc.tensor.matmul(pg[:, u, :], expk_b[:, p * P:(p + 1) * P],
                                 vext_b[:, p, :], start=True, stop=True)
            nc.vector.tensor_add(out=gacc[:, 2 * i:2 * i + 2, :],
                                 in0=gacc[:, 2 * i:2 * i + 2, :], in1=pg[:])

    # ================= AllReduce G within batch pairs ====================
    g_in = dram.tile([P, NPAIR, 130], F32)
    g_out = dram.tile([P, NPAIR, 130], F32)
    nc.gpsimd.dma_start(out=g_in[:], in_=gacc[:])
    nc.gpsimd.collective_compute(
        "AllReduce", mybir.AluOpType.add,
        replica_groups=[[0, 1], [2, 3], [4, 5], [6, 7]],
        ins=[g_in.opt()], outs=[g_out.opt()],
    )
    gall = gacc  # reuse the accumulator tile for the reduced result
    nc.gpsimd.dma_start(out=gall[:], in_=g_out[:])

    # gs[j*64:(j+1)*64, p, :] = per-head g (64x64), scaled by 1/colsum
    rcs = gpool.tile([P, NPAIR], F32)
    nc.vector.reciprocal(out=rcs[:], in_=gall[:, :, 128])
    # block-diagonal per-pair g (off-diagonal cross-head blocks zeroed) so
    # each pair's o needs ONE full-base matmul: two matmuls into the same
    # PSUM bank with mismatched tile_position row bases hang the device.
    g_bd = gpool.tile([P, NPAIR, P], BF16)
    nc.vector.memset(g_bd[:], 0.0)
    for p in range(NPAIR):
        nc.vector.tensor_scalar_mul(out=g_bd[0:64, p, 0:64],
                                    in0=gall[0:64, p, 0:64],
                                    scalar1=rcs[0:64, p:p + 1])
        nc.vector.tensor_scalar_mul(out=g_bd[64:128, p, 64:128],
                                    in0=gall[64:128, p, 64:128],
                                    scalar1=rcs[64:128, p:p + 1])

    # ====== sweep 2: q-projection, o = softmax(q) @ g, residual, LN ======
    INV_N = 1.0 / DM
    for cb in range(NCHUNK):
        # q-projection for this chunk (kept out of sweep 1 so sweep-2 has
        # dense TensorE work alongside its DVE-heavy epilogue)
        for m in range(NKT):
            pq = ps_q.tile([P, CHUNK], F32, tag="pq", name="pq")
            for t in range(NKT):
                nc.tensor.matmul(pq[:], wq[t][:, m * P:(m + 1) * P],
                                 xt[t][cb][:], start=(t == 0),
                                 stop=(t == NKT - 1))
            if has_bq:
                nc.scalar.activation(out=eq[cb][m][:], in_=pq[:],
                                     func=mybir.ActivationFunctionType.Exp,
                                     bias=bq_t[:, m:m + 1])
            else:
                nc.scalar.activation(out=eq[cb][m][:], in_=pq[:],
                                     func=mybir.ActivationFunctionType.Exp)

        for j in range(BPC):
            b = cb * BPC + j
            c = cb
            js = slice(j * P, (j + 1) * P)

            x_b = x2pool.tile([P, DM], F32, tag="x2b", name="x2b")
            nc.sync.dma_start(out=x_b[:], in_=x_d[b * P:(b + 1) * P, :])

            # q-softmax denominator: sum_d exp(q) via ones-masked matmul
            pqd = ps_g.tile([P, H], F32, tag="pg", name="pqd")
            for m in range(NKT):
                nc.tensor.matmul(pqd[:, 2 * m:2 * m + 2], eq[c][m][:, js],
                                 hmask[:], start=True, stop=True)
            rq = smpool.tile([P, H], F32, tag="rq", name="rq")
            nc.vector.reciprocal(out=rq[:], in_=pqd[:])

            # o matmuls per pair; divide by the q-softmax denominator during
            # eviction (inner step-0 AP broadcasts 1/qden over head_dim)
            o_b = opool.tile([P, DM], F32, tag="ob", name="ob")
            for p in range(NPAIR):
                po = ps_v.tile([P, 2, HD], F32, tag="pv", name="po")
                nc.tensor.matmul(po[:].rearrange("p a b -> p (a b)"),
                                 eq[c][p][:, js], g_bd[:, p, :],
                                 start=True, stop=True)
                rqs = rq[:, 2 * p:2 * p + 2]
                rq_bc = bass.AP(tensor=rqs.tensor, offset=rqs.offset,
                                ap=list(rqs.ap) + [[0, HD]])
                nc.vector.tensor_mul(
                    out=o_b[:, p * P:(p + 1) * P].rearrange(
                        "p (h d) -> p h d", h=2),
                    in0=po[:], in1=rq_bc)

            # y = x + o (fused with the channel-sum for the LN mean)
            mv = smpool.tile([P, 4], F32, tag="mv", name="mv")
            nc.vector.tensor_add(out=o_b[:], in0=o_b[:], in1=x_b[:])
            # sum(y^2) on the Scalar engine (squares land in a scratch tile)
            # channel sums for LN mean/var, both on the Scalar engine
            ysq = opool.tile([P, DM], F32, tag="ysq", name="ysq", bufs=2)
            nc.scalar.activation(out=ysq[:], in_=o_b[:],
                                 func=mybir.ActivationFunctionType.Identity,
                                 accum_out=mv[:, 0:1])
            nc.scalar.activation(out=ysq[:], in_=o_b[:],
                                 func=mybir.ActivationFunctionType.Square,
                                 accum_out=mv[:, 1:2])
            # mean = ysum/N; var = ysumsq/N - mean^2; rstd = rsqrt(var+eps)
            nc.vector.tensor_scalar_mul(out=mv[:, 0:1], in0=mv[:, 0:1],
                                        scalar1=INV_N)
            nc.vector.tensor_mul(out=mv[:, 2:3], in0=mv[:, 0:1], in1=mv[:, 0:1])
            nc.vector.tensor_scalar(out=mv[:, 1:2], in0=mv[:, 1:2],
                                    scalar1=INV_N, scalar2=mv[:, 2:3],
                                    op0=mybir.AluOpType.mult,
                                    op1=mybir.AluOpType.subtract)
            nc.scalar.activation(out=mv[:, 1:2], in_=mv[:, 1:2],
                                 func=mybir.ActivationFunctionType.Sqrt,
                                 bias=eps_t[:])
            nc.vector.reciprocal(out=mv[:, 1:2], in_=mv[:, 1:2])
            nc.vector.tensor_scalar(out=o_b[:], in0=o_b[:],
                                    scalar1=mv[:, 0:1], scalar2=mv[:, 1:2],
                                    op0=mybir.AluOpType.subtract,
                                    op1=mybir.AluOpType.mult)
            if has_gamma:
                nc.vector.tensor_mul(out=o_b[:], in0=o_b[:], in1=gamma_bc[:])
            if has_beta:
                nc.vector.tensor_add(out=o_b[:], in0=o_b[:], in1=beta_bc[:])
            nc.sync.dma_start(out=out_d[b * P:(b + 1) * P, :], in_=o_b[:])


_PROGRAM_CACHE = {}


def _build_program(flags):
    if flags in _PROGRAM_CACHE:
        return _PROGRAM_CACHE[flags]
    nc = bass.Bass("TRN2", target_bir_lowering=False, debug=False,
                   num_devices=NCORES)
    x_d = nc.dram_tensor("x_shard", [R, DM], F32, kind="ExternalInput").ap()
    xt_d = nc.dram_tensor("xt_shard", [DM, R], F32, kind="ExternalInput").ap()
    wqt_d = nc.dram_tensor("wq_t", [DM, DM], F32, kind="ExternalInput").ap()
    wkt_d = nc.dram_tensor("wk_t", [DM, DM], F32, kind="ExternalInput").ap()
    wvt_d = nc.dram_tensor("wv_t", [DM, DM], F32, kind="ExternalInput").ap()
    bq_d = nc.dram_tensor("bq", [DM], F32, kind="ExternalInput").ap()
    bk_d = nc.dram_tensor("bk", [DM], F32, kind="ExternalInput").ap()
    bv_d = nc.dram_tensor("bv", [DM], F32, kind="ExternalInput").ap()
    gamma_d = nc.dram_tensor("gamma", [DM], F32, kind="ExternalInput").ap()
    beta_d = nc.dram_tensor("beta", [DM], F32, kind="ExternalInput").ap()
    out_d = nc.dram_tensor("out_shard", [R, DM], F32, kind="ExternalOutput").ap()
    io = (x_d, xt_d, wqt_d, wkt_d, wvt_d, bq_d, bk_d, bv_d, gamma_d, beta_d,
          out_d)
    with tile.TileContext(nc) as tc:
        with ExitStack() as ctx:
            _body(ctx, tc, io, flags)
    _fix_multiwaits(nc)
    _PROGRAM_CACHE[flags] = nc
    return nc


def kernel(x, mask, pad_mask, Wq, bq, Wk, bk, Wv, bv, gamma, beta):
    x = np.ascontiguousarray(np.asarray(x, dtype=np.float32))
    flags = (bool(np.any(bq)), bool(np.any(bk)), bool(np.any(bv)),
             bool(np.any(np.asarray(gamma) != 1.0)), bool(np.any(beta)))
    nc = _build_program(flags)

    common = {
        "wq_t": np.ascontiguousarray(np.asarray(Wq, dtype=np.float32).T),
        "wk_t": np.ascontiguousarray(np.asarray(Wk, dtype=np.float32).T),
        "wv_t": np.ascontiguousarray(np.asarray(Wv, dtype=np.float32).T),
        "bq": np.ascontiguousarray(bq, dtype=np.float32),
        "bk": np.ascontiguousarray(bk, dtype=np.float32),
        "bv": np.ascontiguousarray(bv, dtype=np.float32),
        "gamma": np.ascontiguousarray(gamma, dtype=np.float32),
        "beta": np.ascontiguousarray(beta, dtype=np.float32),
    }
    in_maps = []
    for c in range(NCORES):
        b, half = divmod(c, 2)
        shard = np.ascontiguousarray(x[b, half * R:(half + 1) * R, :])
        in_maps.append({"x_shard": shard,
                        "xt_shard": np.ascontiguousarray(shard.T),
                        **common})

    res = run_bass_kernel_spmd(nc, in_maps, list(range(NCORES)))

    out = np.empty((B, S, DM), dtype=np.float32)
    for c in range(NCORES):
        b, half = divmod(c, 2)
        out[b, half * R:(half + 1) * R, :] = res.results[c]["out_shard"]
    return out


if __name__ == "__main__":
    rng = np.random.default_rng(0)
    demo = {
        "x": rng.standard_normal((B, S, DM), dtype=np.float32),
        "mask": np.zeros((S, S), bool),
        "pad_mask": np.zeros((B, S), bool),
        "Wq": rng.uniform(-0.03, 0.03, (DM, DM)).astype(np.float32),
        "bq": np.zeros(DM, np.float32),
        "Wk": rng.uniform(-0.03, 0.03, (DM, DM)).astype(np.float32),
        "bk": np.zeros(DM, np.float32),
        "Wv": rng.uniform(-0.03, 0.03, (DM, DM)).astype(np.float32),
        "bv": np.zeros(DM, np.float32),
        "gamma": np.ones(DM, np.float32),
        "beta": np.zeros(DM, np.float32),
    }
    out = kernel(**demo)
    print("out", out.shape, out.dtype, float(np.abs(out).max()))


# revision 26
# speedup vs baseline: 1.8297x; 1.1023x over previous
"""Trainium2 Bass kernel for nn_MultiHeadAttention_Linear_11312943857747.

Math (B=4, S=4096, DM=1024, H=16, HD=64):
    q = softmax(x @ Wq.T + bq) over head_dim
    k = softmax(x @ Wk.T + bk) over seq_len
    v = x @ Wv.T + bv
    gmap[b,h] = k[b,h].T @ v[b,h]            (HD x HD per head)
    o[b,h]    = q[b,h] @ gmap[b,h]
    out = LayerNorm(x + o) * gamma + beta

Sharding: 8 cores = 4 batches x 2 sequence-halves. Each core projects its
2048 rows; the per-head kT@v reduction over the full sequence is completed
with a tiny (132KB) AllReduce between the two cores sharing a batch
(replica groups [[0,1],[2,3],[4,5],[6,7]]).

Both softmaxes are folded into matmuls:
  - k-softmax over S: gmap = (exp(k)/colsum).T @ v
      == diag(1/colsum) @ (exp(k).T @ [v | 1]); the ones column produces
      colsum in the same PSUM accumulation, and the divide happens after
      the AllReduce on the 64x65-per-head global map.
  - q-softmax over HD: o = (exp(q) @ g) / rowsum(exp(q)); rowsum comes from
      a ones-masked matmul and the divide is fused into PSUM eviction.

exp() needs no max-subtraction here: k,q = x @ W.T with |entries| <~ 5, and
softmax is shift-invariant so the result matches the reference exactly up
to fp rounding.

Matmul operands are cast to bf16 on-chip (PSUM accumulation stays fp32):
fp32/fp32r matmuls run as two half-rate passes on TRN2, bf16 single-pass.
The host supplies x in both layouts (x and x.T) so no PE transposes are
needed; the contraction layout is required by the TensorE (stationary and
moving operands both index the contraction dim on partitions).
"""

import sys

sys.path.insert(0, "/opt/trn_rl_repo")

import numpy as np
from contextlib import ExitStack

import concourse.bass as bass
import concourse.mybir as mybir
import concourse.tile as tile
from concourse.bass_utils import run_bass_kernel_spmd

F32 = mybir.dt.float32
BF16 = mybir.dt.bfloat16

B, S, DM, H, HD = 4, 4096, 1024, 16, 64
EPS = 1e-5
NCORES = 8
R = S // 2          # rows per core
P = 128             # partitions
NBLK = R // P       # 16 sequence blocks of 128 rows
NKT = DM // P       # 8 k-tiles over the contraction dim
NPAIR = DM // P     # 8 head-pairs (2 heads of 64 = 128 channels)
CHUNK = 512         # moving-operand width for the big projections
NCHUNK = R // CHUNK # 4
BPC = CHUNK // P    # 4 blocks per chunk


def _fix_multiwaits(nc):
    """This walrus build encodes at most one sync wait per instruction;
    split any multi-wait instruction into preceding same-engine NoOps."""
    for fn in nc.m.functions:
        for bb in fn.blocks:
            new_insts = []
            changed = False
            for ins in bb.instructions:
                si = ins.sync_info
                if si is not None and si.on_wait and len(si.on_wait) > 1:
                    waits = list(si.on_wait)
                    for i, w in enumerate(waits[:-1]):
                        new_insts.append(
                            mybir.InstNoOp(
                                name=f"{ins.name}-wsplit{i}",
                                engine=ins.engine,
                                sync_info=mybir.SyncInfo(on_wait=[w], on_update=[]),
                                bass_nofuse=True,
                            )
                        )
                    ins.sync_info = mybir.SyncInfo(
                        on_wait=[waits[-1]], on_update=list(si.on_update or [])
                    )
                    changed = True
                new_insts.append(ins)
            if changed:
                bb.instructions = new_insts


def _body(ctx, tc, io, flags):
    nc = tc.nc
    has_bq, has_bk, has_bv, has_gamma, has_beta = flags
    (x_d, xt_d, wqt_d, wkt_d, wvt_d, bq_d, bk_d, bv_d, gamma_d, beta_d,
     out_d) = io

    const = ctx.enter_context(tc.tile_pool(name="const", bufs=1))
    wpool = ctx.enter_context(tc.tile_pool(name="w", bufs=1))
    stgpool = ctx.enter_context(tc.tile_pool(name="stg", bufs=6))
    xtpool = ctx.enter_context(tc.tile_pool(name="xt", bufs=1))
    x2pool = ctx.enter_context(tc.tile_pool(name="x2", bufs=8))
    kvpool = ctx.enter_context(tc.tile_pool(name="kv", bufs=3))
    eqpool = ctx.enter_context(tc.tile_pool(name="eq", bufs=1))
    opool = ctx.enter_context(tc.tile_pool(name="o", bufs=3))
    gpool = ctx.enter_context(tc.tile_pool(name="g", bufs=1))
    smpool = ctx.enter_context(tc.tile_pool(name="sm", bufs=3))
    dram = ctx.enter_context(tc.tile_pool(name="dram", bufs=1, space="DRAM"))

    ps_k = ctx.enter_context(tc.tile_pool(name="ps_k", bufs=2, space="PSUM"))
    ps_v = ctx.enter_context(tc.tile_pool(name="ps_v", bufs=2, space="PSUM"))
    ps_g = ctx.enter_context(tc.tile_pool(name="ps_g", bufs=2, space="PSUM"))
    ps_q = ctx.enter_context(tc.tile_pool(name="ps_q", bufs=2, space="PSUM"))
    # sweep 2 reuses the budget: qden shares ps_g, o-matmuls share ps_v.

    # ---- constants -----------------------------------------------------
    eps_t = const.tile([P, 1], F32)
    nc.vector.memset(eps_t[:], EPS)

    # ones-mask [128, 2]: col j selects the 64 partitions of head j in a pair
    hmask = const.tile([P, 2], BF16)
    nc.vector.memset(hmask[:], 0.0)
    nc.vector.memset(hmask[0:64, 0:1], 1.0)
    nc.vector.memset(hmask[64:128, 1:2], 1.0)

    # ---- weights + x.T: DMA fp32 staging -> bf16 tiles -----------------
    # Order matters for the startup critical path: x.T chunk 0 and Wk/Wv
    # feed the first projections; Wq and the other chunks can trail.
    wq = [wpool.tile([P, DM], BF16, tag=f"wq{t}", name=f"wq{t}") for t in range(NKT)]
    wk = [wpool.tile([P, DM], BF16, tag=f"wk{t}", name=f"wk{t}") for t in range(NKT)]
    wv = [wpool.tile([P, DM], BF16, tag=f"wv{t}", name=f"wv{t}") for t in range(NKT)]
    xt = [[xtpool.tile([P, CHUNK], BF16, tag=f"xt{t}_{c}", name=f"xt{t}_{c}")
           for c in range(NCHUNK)] for t in range(NKT)]

    def _load_xt_chunk(c):
        for t in range(NKT):
            stg = stgpool.tile([P, CHUNK], F32, tag="stg", name="xtstg")
            nc.sync.dma_start(
                out=stg[:],
                in_=xt_d[t * P:(t + 1) * P, c * CHUNK:(c + 1) * CHUNK])
            nc.vector.tensor_copy(out=xt[t][c][:], in_=stg[:])

    def _load_w(dst, src_d):
        for t in range(NKT):
            stg = stgpool.tile([P, DM], F32, tag="stg", name="wstg")
            nc.sync.dma_start(out=stg[:], in_=src_d[t * P:(t + 1) * P, :])
            nc.vector.tensor_copy(out=dst[t][:], in_=stg[:])

    _load_xt_chunk(0)
    _load_w(wk, wkt_d)
    _load_w(wv, wvt_d)
    for c in range(1, NCHUNK):
        _load_xt_chunk(c)
    _load_w(wq, wqt_d)

    bq_t = None
    if has_bq:
        bq_t = const.tile([P, NKT], F32)
        nc.sync.dma_start(out=bq_t[:], in_=bq_d.rearrange("(t p) -> p t", p=P))
    bk_bc = bv_bc = gamma_bc = beta_bc = None

    def _bcast(src_d):
        t = const.tile([P, DM], F32, name=f"bc_{src_d.tensor.name}")
        src = bass.AP(tensor=src_d.tensor, offset=src_d.offset,
                      ap=[[0, P]] + list(src_d.ap))
        nc.sync.dma_start(out=t[:], in_=src)
        return t

    if has_bk:
        bk_bc = _bcast(bk_d)
    if has_bv:
        bv_bc = _bcast(bv_d)
    if has_gamma:
        gamma_bc = _bcast(gamma_d)
    if has_beta:
        beta_bc = _bcast(beta_d)

    # G accumulator [128, pair, 129]: cols 0..127 = 2-head block of kT@v
    # (only the two diagonal 64x64 blocks are meaningful), col 128 = colsum.
    gacc = gpool.tile([P, NPAIR, 130], F32)
    nc.vector.memset(gacc[:], 0.0)

    # eq[c][m]: exp(q).T for chunk c, channel tile m — bf16, all resident
    eq = [[eqpool.tile([P, CHUNK], BF16, tag=f"eq{c}_{m}", name=f"eq{c}_{m}")
           for m in range(NKT)] for c in range(NCHUNK)]

    # ============ sweep 1: k/v/q projections + G accumulation ===========
    for b in range(NBLK):
        c, j = divmod(b, BPC)
        js = slice(j * P, (j + 1) * P)

        expk_b = kvpool.tile([P, DM], BF16, tag="ek", name="ek")
        vext_b = kvpool.tile([P, NPAIR, 130], BF16, tag="vx", name="vx")
        nc.vector.memset(vext_b[:, :, 128:130], 1.0)
        for cc in range(2):
            cs = slice(cc * CHUNK, (cc + 1) * CHUNK)
            pk = ps_k.tile([P, CHUNK], F32, tag="pk", name="pk")
            pv = ps_v.tile([P, CHUNK], F32, tag="pv", name="pv")
            for t in range(NKT):
                lhsT = xt[t][c][:, js]
                nc.tensor.matmul(pk[:], lhsT, wk[t][:, cs],
                                 start=(t == 0), stop=(t == NKT - 1))
                nc.tensor.matmul(pv[:], lhsT, wv[t][:, cs],
                                 start=(t == 0), stop=(t == NKT - 1))
            if has_bk:
                nc.vector.tensor_add(out=expk_b[:, cs], in0=pk[:], in1=bk_bc[:, cs])
                nc.scalar.activation(out=expk_b[:, cs], in_=expk_b[:, cs],
                                     func=mybir.ActivationFunctionType.Exp)
            else:
                nc.scalar.activation(out=expk_b[:, cs], in_=pk[:],
                                     func=mybir.ActivationFunctionType.Exp)
            vdst = vext_b[:, 4 * cc:4 * (cc + 1), 0:128]
            psrc = pv[:].rearrange("p (a b) -> p a b", a=4)
            if has_bv:
                nc.vector.tensor_add(
                    out=vdst, in0=psrc,
                    in1=bv_bc[:, cs].rearrange("p (a b) -> p a b", a=4))
            else:
                nc.vector.tensor_copy(out=vdst, in_=psrc)

        # G += expk_pair.T @ [v_pair | 1]; two pairs share one PSUM bank
        for i in range(NPAIR // 2):
            pg = ps_g.tile([P, 2, 130], F32, tag="pg", name="pg")
            for u in range(2):
                p = 2 * i + u
                nc.tensor.matmul(pg[:, u, :], expk_b[:, p * P:(p + 1) * P],
                                 vext_b[:, p, :], start=True, stop=True)
            nc.vector.tensor_add(out=gacc[:, 2 * i:2 * i + 2, :],
                                 in0=gacc[:, 2 * i:2 * i + 2, :], in1=pg[:])

    # ================= AllReduce G within batch pairs ====================
    g_in = dram.tile([P, NPAIR, 130], F32)
    g_out = dram.tile([P, NPAIR, 130], F32)
    nc.gpsimd.dma_start(out=g_in[:], in_=gacc[:])
    nc.gpsimd.collective_compute(
        "AllReduce", mybir.AluOpType.add,
        replica_groups=[[0, 1], [2, 3], [4, 5], [6, 7]],
        ins=[g_in.opt()], outs=[g_out.opt()],
    )
    gall = gacc  # reuse the accumulator tile for the reduced result
    nc.gpsimd.dma_start(out=gall[:], in_=g_out[:])

    # gs[j*64:(j+1)*64, p, :] = per-head g (64x64), scaled by 1/colsum
    rcs = gpool.tile([P, NPAIR], F32)
    nc.vector.reciprocal(out=rcs[:], in_=gall[:, :, 128])
    # block-diagonal per-pair g (off-diagonal cross-head blocks zeroed) so
    # each pair's o needs ONE full-base matmul: two matmuls into the same
    # PSUM bank with mismatched tile_position row bases hang the device.
    g_bd = gpool.tile([P, NPAIR, P], BF16)
    nc.vector.memset(g_bd[:], 0.0)
    for p in range(NPAIR):
        nc.vector.tensor_scalar_mul(out=g_bd[0:64, p, 0:64],
                                    in0=gall[0:64, p, 0:64],
                                    scalar1=rcs[0:64, p:p + 1])
        nc.vector.tensor_scalar_mul(out=g_bd[64:128, p, 64:128],
                                    in0=gall[64:128, p, 64:128],
                                    scalar1=rcs[64:128, p:p + 1])

    # ====== sweep 2: q-projection, o = softmax(q) @ g, residual, LN ======
    INV_N = 1.0 / DM
    for cb in range(NCHUNK):
        # q-projection for this chunk (kept out of sweep 1 so sweep-2 has
        # dense TensorE work alongside its DVE-heavy epilogue)
        for m in range(NKT):
            pq = ps_q.tile([P, CHUNK], F32, tag="pq", name="pq")
            for t in range(NKT):
                nc.tensor.matmul(pq[:], wq[t][:, m * P:(m + 1) * P],
                                 xt[t][cb][:], start=(t == 0),
                                 stop=(t == NKT - 1))
            if has_bq:
                nc.scalar.activation(out=eq[cb][m][:], in_=pq[:],
                                     func=mybir.ActivationFunctionType.Exp,
                                     bias=bq_t[:, m:m + 1])
            else:
                nc.scalar.activation(out=eq[cb][m][:], in_=pq[:],
                                     func=mybir.ActivationFunctionType.Exp)

        for j in range(BPC):
            b = cb * BPC + j
            c = cb
            js = slice(j * P, (j + 1) * P)

            x_b = x2pool.tile([P, DM], F32, tag="x2b", name="x2b")
            nc.sync.dma_start(out=x_b[:], in_=x_d[b * P:(b + 1) * P, :])

            # q-softmax denominator: sum_d exp(q) via ones-masked matmul
            pqd = ps_g.tile([P, H], F32, tag="pg", name="pqd")
            for m in range(NKT):
                nc.tensor.matmul(pqd[:, 2 * m:2 * m + 2], eq[c][m][:, js],
                                 hmask[:], start=True, stop=True)
            rq = smpool.tile([P, H], F32, tag="rq", name="rq")
            nc.vector.reciprocal(out=rq[:], in_=pqd[:])

            # o matmuls per pair; divide by the q-softmax denominator during
            # eviction (inner step-0 AP broadcasts 1/qden over head_dim)
            o_b = opool.tile([P, DM], F32, tag="ob", name="ob")
            for p in range(NPAIR):
                po = ps_v.tile([P, 2, HD], F32, tag="pv", name="po")
                nc.tensor.matmul(po[:].rearrange("p a b -> p (a b)"),
                                 eq[c][p][:, js], g_bd[:, p, :],
                                 start=True, stop=True)
                rqs = rq[:, 2 * p:2 * p + 2]
                rq_bc = bass.AP(tensor=rqs.tensor, offset=rqs.offset,
                                ap=list(rqs.ap) + [[0, HD]])
                nc.vector.tensor_mul(
                    out=o_b[:, p * P:(p + 1) * P].rearrange(
                        "p (h d) -> p h d", h=2),
                    in0=po[:], in1=rq_bc)

            # y = x + o (fused with the channel-sum for the LN mean)
            mv = smpool.tile([P, 4], F32, tag="mv", name="mv")
            nc.vector.tensor_add(out=o_b[:], in0=o_b[:], in1=x_b[:])
            # sum(y^2) on the Scalar engine (squares land in a scratch tile)
            # channel sums for LN mean/var, both on the Scalar engine
            ysq = opool.tile([P, DM], F32, tag="ysq", name="ysq", bufs=2)
            nc.scalar.activation(out=ysq[:], in_=o_b[:],
                                 func=mybir.ActivationFunctionType.Identity,
                                 accum_out=mv[:, 0:1])
            nc.scalar.activation(out=ysq[:], in_=o_b[:],
                                 func=mybir.ActivationFunctionType.Square,
                                 accum_out=mv[:, 1:2])
            # mean = ysum/N; var = ysumsq/N - mean^2; rstd = rsqrt(var+eps)
            nc.vector.tensor_scalar_mul(out=mv[:, 0:1], in0=mv[:, 0:1],
                                        scalar1=INV_N)
            nc.vector.tensor_mul(out=mv[:, 2:3], in0=mv[:, 0:1], in1=mv[:, 0:1])
            nc.vector.tensor_scalar(out=mv[:, 1:2], in0=mv[:, 1:2],
                                    scalar1=INV_N, scalar2=mv[:, 2:3],
                                    op0=mybir.AluOpType.mult,
                                    op1=mybir.AluOpType.subtract)
            nc.scalar.activation(out=mv[:, 1:2], in_=mv[:, 1:2],
                                 func=mybir.ActivationFunctionType.Sqrt,
                                 bias=eps_t[:])
            nc.vector.reciprocal(out=mv[:, 1:2], in_=mv[:, 1:2])
            nc.vector.tensor_scalar(out=o_b[:], in0=o_b[:],
                                    scalar1=mv[:, 0:1], scalar2=mv[:, 1:2],
                                    op0=mybir.AluOpType.subtract,
                                    op1=mybir.AluOpType.mult)
            if has_gamma:
                nc.vector.tensor_mul(out=o_b[:], in0=o_b[:], in1=gamma_bc[:])
            if has_beta:
                nc.vector.tensor_add(out=o_b[:], in0=o_b[:], in1=beta_bc[:])
            nc.sync.dma_start(out=out_d[b * P:(b + 1) * P, :], in_=o_b[:])


_PROGRAM_CACHE = {}


def _build_program(flags):
    if flags in _PROGRAM_CACHE:
        return _PROGRAM_CACHE[flags]
    nc = bass.Bass("TRN2", target_bir_lowering=False, debug=False,
                   num_devices=NCORES)
    x_d = nc.dram_tensor("x_shard", [R, DM], F32, kind="ExternalInput").ap()
    xt_d = nc.dram_tensor("xt_shard", [DM, R], F32, kind="ExternalInput").ap()
    wqt_d = nc.dram_tensor("wq_t", [DM, DM], F32, kind="ExternalInput").ap()
    wkt_d = nc.dram_tensor("wk_t", [DM, DM], F32, kind="ExternalInput").ap()
    wvt_d = nc.dram_tensor("wv_t", [DM, DM], F32, kind="ExternalInput").ap()
    bq_d = nc.dram_tensor("bq", [DM], F32, kind="ExternalInput").ap()
    bk_d = nc.dram_tensor("bk", [DM], F32, kind="ExternalInput").ap()
    bv_d = nc.dram_tensor("bv", [DM], F32, kind="ExternalInput").ap()
    gamma_d = nc.dram_tensor("gamma", [DM], F32, kind="ExternalInput").ap()
    beta_d = nc.dram_tensor("beta", [DM], F32, kind="ExternalInput").ap()
    out_d = nc.dram_tensor("out_shard", [R, DM], F32, kind="ExternalOutput").ap()
    io = (x_d, xt_d, wqt_d, wkt_d, wvt_d, bq_d, bk_d, bv_d, gamma_d, beta_d,
          out_d)
    with tile.TileContext(nc) as tc:
        with ExitStack() as ctx:
            _body(ctx, tc, io, flags)
    _fix_multiwaits(nc)
    _PROGRAM_CACHE[flags] = nc
    return nc


def kernel(x, mask, pad_mask, Wq, bq, Wk, bk, Wv, bv, gamma, beta):
    x = np.ascontiguousarray(np.asarray(x, dtype=np.float32))
    flags = (bool(np.any(bq)), bool(np.any(bk)), bool(np.any(bv)),
             bool(np.any(np.asarray(gamma) != 1.0)), bool(np.any(beta)))
    nc = _build_program(flags)

    common = {
        "wq_t": np.ascontiguousarray(np.asarray(Wq, dtype=np.float32).T),
        "wk_t": np.ascontiguousarray(np.asarray(Wk, dtype=np.float32).T),
        "wv_t": np.ascontiguousarray(np.asarray(Wv, dtype=np.float32).T),
        "bq": np.ascontiguousarray(bq, dtype=np.float32),
        "bk": np.ascontiguousarray(bk, dtype=np.float32),
        "bv": np.ascontiguousarray(bv, dtype=np.float32),
        "gamma": np.ascontiguousarray(gamma, dtype=np.float32),
        "beta": np.ascontiguousarray(beta, dtype=np.float32),
    }
    in_maps = []
    for c in range(NCORES):
        b, half = divmod(c, 2)
        shard = np.ascontiguousarray(x[b, half * R:(half + 1) * R, :])
        in_maps.append({"x_shard": shard,
                        "xt_shard": np.ascontiguousarray(shard.T),
                        **common})

    res = run_bass_kernel_spmd(nc, in_maps, list(range(NCORES)))

    out = np.empty((B, S, DM), dtype=np.float32)
    for c in range(NCORES):
        b, half = divmod(c, 2)
        out[b, half * R:(half + 1) * R, :] = res.results[c]["out_shard"]
    return out


if __name__ == "__main__":
    rng = np.random.default_rng(0)
    demo = {
        "x": rng.standard_normal((B, S, DM), dtype=np.float32),
        "mask": np.zeros((S, S), bool),
        "pad_mask": np.zeros((B, S), bool),
        "Wq": rng.uniform(-0.03, 0.03, (DM, DM)).astype(np.float32),
        "bq": np.zeros(DM, np.float32),
        "Wk": rng.uniform(-0.03, 0.03, (DM, DM)).astype(np.float32),
        "bk": np.zeros(DM, np.float32),
        "Wv": rng.uniform(-0.03, 0.03, (DM, DM)).astype(np.float32),
        "bv": np.zeros(DM, np.float32),
        "gamma": np.ones(DM, np.float32),
        "beta": np.zeros(DM, np.float32),
    }
    out = kernel(**demo)
    print("out", out.shape, out.dtype, float(np.abs(out).max()))
